# revision 29
# baseline (speedup 1.0000x reference)
"""Trainium2 Bass kernel for the attention-scoring MLP (nn_Attn):

    enc = encoder_outputs.transpose(1,0,2)          # [B,S,Hin]
    a1  = tanh(enc @ W1_enc.T + hidden @ W1_hid.T + b1)
    s   = a1 @ W2[0] (+ b2 -- dropped: softmax shift-invariant)
    s   = where(mask, -inf, s)
    out = softmax(s, axis=-1)[:, None, :]           # [B,1,S]

Strategy (v6):
  * Data-parallel over batch B=32 across 8 NeuronCores (4 rows each),
    weights replicated, no collectives.
  * Mask packing: masked positions get attn == 0 exactly, so only the
    ~50% unmasked columns of enc are shipped/computed. Host packs each
    row's unmasked columns; device computes scores+softmax on the packed
    stream; host scatters back to [B,1,S] with zeros. Geometry: each row
    contributes a MAIN-wide stream (<=448) plus its overflow columns;
    the 4 rows' overflows are batched into shared overflow streams so
    every matmul keeps free-dim >= 256 (tiny-FD matmuls are LDWEIGHTS-
    bound and waste the PE).
  * Everything on the PE runs as fp8 (e4m3) DoubleRow matmuls -- two
    128-deep k-slices per instruction, 2x bf16 throughput, and no
    perf-mode switches (each switch costs a pipeline flush):
      - enc @ W1_enc.T: enc x32, W1 x2^13 host-side (clears fp8
        subnormals); the 2^-18 compensation rides the tanh scale port.
      - scores: tanh outputs are written as fp8; W2 is scaled x2^12 and
        contracted over ht-PAIRS (stationary [128,2,128], col 0 = w2);
        the 2^-12 compensation rides the exp scale port.
    The hidden @ W1_hid.T + b1 term (0.1% of FLOPs) is folded host-side
    into the per-(h,b) tanh bias: row streams use the activation's
    per-partition bias port; overflow streams (mixed rows per tile) get
    a host-precomputed broadcast bias added on the otherwise-idle DVE,
    pre-scaled by 2^18 so the tanh scale still matches.
  * Softmax per row on-device: exp(2^-12 s - 40) with accumulate,
    reciprocal, scale, DMA out the packed attn row.
"""

import numpy as np
import ml_dtypes

import concourse.bass as bass
import concourse.tile as tile
from concourse import bacc, mybir
from concourse.bass import ds
from concourse.bass_utils import run_bass_kernel_spmd

N_CORES = 8
B, S, HIN, H = 32, 1024, 1024, 1024
BL = B // N_CORES          # local batch rows per core
P = 128                    # partitions
IT = HIN // P              # 128-deep contraction tiles
IT2 = IT // 2              # DoubleRow pair tiles
HT = H // P                # output-feature tiles
HTP = HT // 2              # ht pairs for the DR scores contraction
F32 = mybir.dt.float32
BF16 = mybir.dt.bfloat16
F8 = mybir.dt.float8e4
AF = mybir.ActivationFunctionType
DR = mybir.MatmulPerfMode.DoubleRow
BF = ml_dtypes.bfloat16
F8NP = ml_dtypes.float8_e4m3

USE_FP8 = True
ENC_SCALE = 32.0
W_SCALE = float(2.0 ** 13)
ACT_SCALE = float(1.0 / (ENC_SCALE * W_SCALE)) if USE_FP8 else 1.0
W2_SCALE = float(2.0 ** 12) if USE_FP8 else 1.0
DELAY = 2                  # pending tanh->scores pipeline depth (ht pairs)

_cached = {}               # (main, rem) -> compiled Bacc
LAST_RESULT = None         # BassKernelResults of the most recent run


def _geometry(lmax: int):
    """MAIN width per row plus overflow-block widths (multiples of 16)."""
    rnd = lambda x: ((x + 15) // 16) * 16
    if lmax <= 448:
        return rnd(lmax), ()
    rem = []
    off = 448
    while off < lmax:
        rem.append(min(128, rnd(lmax - off)))
        off += rem[-1]
    return 448, tuple(rem)


def _build(main: int, rem: tuple):
    key = (main, rem)
    if key in _cached:
        return _cached[key]

    LT = main + sum(rem)
    EDT = F8 if USE_FP8 else BF16
    SDT = F8 if USE_FP8 else BF16     # scores-matmul operand dtype

    nc = bacc.Bacc("TRN2", target_bir_lowering=False, debug=False,
                   num_devices=N_CORES)

    encm_ext = nc.dram_tensor("encm", [BL, HIN, main], EDT, kind="ExternalInput").ap()
    encr_ext = [
        nc.dram_tensor(f"encr{k}", [HIN, BL * rk], EDT, kind="ExternalInput").ap()
        for k, rk in enumerate(rem)
    ]
    w1_ext = nc.dram_tensor("w1p", [P, HT * IT * P], EDT, kind="ExternalInput").ap()
    bias_ext = nc.dram_tensor("biasT", [P, HT * BL], F32, kind="ExternalInput").ap()
    # overflow broadcast bias, pre-scaled by 1/ACT_SCALE (bf16: half the
    # wire bytes; the DVE add upcasts)
    biasb_ext = [
        nc.dram_tensor(f"biasb{k}", [P, HT * BL * rk], BF16, kind="ExternalInput").ap()
        for k, rk in enumerate(rem)
    ]
    # W2 stationary: fp8 ht-pairs [p, htp, two, m] (col 0 = w2*2^12), or
    # bf16 [p, ht, m] for the fallback path
    w2_ext = nc.dram_tensor("w2pad", [P, HT * P], SDT, kind="ExternalInput").ap()
    pneg_ext = nc.dram_tensor("padneg", [BL * LT], F32, kind="ExternalInput").ap()
    # out rows are unnormalized exp(s - 40); osum has the row sums
    # (host divides during the scatter -- keeps recip+mul off the tail)
    out_ext = nc.dram_tensor("out", [BL, LT], F32, kind="ExternalOutput").ap()
    osum_ext = nc.dram_tensor("osum", [BL], F32, kind="ExternalOutput").ap()

    def pair2(ap2d):
        return ap2d.rearrange("p (two m) -> p two m", two=2)

    def wpair(w_sb, ht, it2):
        return pair2(w_sb[:, ds((ht * IT2 + it2) * 2 * P, 2 * P)])

    with tile.TileContext(nc) as tc:
        with (
            tc.tile_pool(name="consts", bufs=1) as consts,
            tc.tile_pool(name="thp", bufs=4) as thp,
            tc.tile_pool(name="pap", bufs=3, space="PSUM") as pap,
            tc.tile_pool(name="parp", bufs=3, space="PSUM") as parp,
            tc.tile_pool(name="pscp", bufs=2, space="PSUM") as pscp,
        ):
            # ---- PE warmup: junk matmuls with no DMA deps so the HAM
            # clock-gate ramps while the first DMAs land.
            warm_sb = consts.tile([P, 512], BF16)
            nc.gpsimd.memset(warm_sb[:], 0.0)
            for _ in range(8):
                warm_ps = pap.tile([P, main], F32, tag="pa")
                nc.tensor.matmul(warm_ps[:], warm_sb[:, 0:P],
                                 warm_sb[:, 0:main], start=True, stop=True)

            # ---- resident weights/constants.
            # Emission order = ring service order: first-needed first.
            # w1 split per ht so the first overflow matmul only waits for
            # one eighth of the 1MB weight load.
            w1_sb = consts.tile([P, HT * IT * P], EDT)
            nc.sync.dma_start(w1_sb[:, ds(0, IT * P)], w1_ext[:, ds(0, IT * P)])
            encr_sb, biasb_sb = [], []
            for k, rk in enumerate(rem):
                w = BL * rk
                e = consts.tile([P, IT, w], EDT, tag=f"encr{k}")
                h = IT // 2
                # halves on different rings: each is descriptor-rate
                # limited, so serializing both on one queue delays the
                # whole overflow phase
                nc.scalar.dma_start(
                    e[:, ds(0, h), :],
                    encr_ext[k][ds(0, h * P), :].rearrange(
                        "(it p) n -> p it n", p=P))
                nc.sync.dma_start(
                    e[:, ds(h, h), :],
                    encr_ext[k][ds(h * P, h * P), :].rearrange(
                        "(it p) n -> p it n", p=P))
                encr_sb.append(e)
                bb = consts.tile([P, HT * w], BF16, tag=f"biasb{k}")
                nc.sync.dma_start(bb[:], biasb_ext[k][:, :])
                biasb_sb.append(bb)
            for ht in range(1, HT):
                nc.sync.dma_start(w1_sb[:, ds(ht * IT * P, IT * P)],
                                  w1_ext[:, ds(ht * IT * P, IT * P)])
            w2_sb = consts.tile([P, HT * P], SDT)
            nc.sync.dma_start(w2_sb[:], w2_ext[:, :])
            pneg_sb = consts.tile([1, BL * LT], F32)
            nc.scalar.dma_start(pneg_sb[:], pneg_ext[:])
            bias_sb = consts.tile([P, HT * BL], F32)
            nc.scalar.dma_start(bias_sb[:], bias_ext[:, :])
            encm_sb = []
            for r in range(BL):
                e = consts.tile([P, IT, main], EDT, tag=f"encm{r}")
                eng = nc.scalar if r < 2 else nc.sync
                eng.dma_start(
                    e[:], encm_ext[r].rearrange("(it p) n -> p it n", p=P))
                encm_sb.append(e)

            scores_sb = consts.tile([1, BL * LT], F32)
            c40 = consts.tile([1, 1], F32)
            nc.gpsimd.memset(c40[:], -40.0)
            exps = consts.tile([1, BL * LT], F32)
            ssum = consts.tile([1, BL], F32)

            def scores_mm(psum, n, thP, htp):
                if USE_FP8:
                    nc.tensor.matmul(psum[:, 0:n],
                                     pair2(w2_sb[:, ds(htp * 2 * P, 2 * P)]),
                                     pair2(thP[:]), start=(htp == 0),
                                     stop=(htp == HTP - 1), perf_mode=DR)
                else:
                    for i in range(2):
                        ht = 2 * htp + i
                        nc.tensor.matmul(psum[:, 0:n],
                                         w2_sb[:, ds(ht * P, P)],
                                         thP[:, ds(i * n, n)],
                                         start=(ht == 0), stop=(ht == HT - 1))

            def main_mms(psum, enc_sb, ht):
                for it2 in range(IT2):
                    if USE_FP8:
                        nc.tensor.matmul(
                            psum[:], wpair(w1_sb, ht, it2),
                            enc_sb[:, ds(2 * it2, 2), :],
                            start=(it2 == 0), stop=(it2 == IT2 - 1),
                            perf_mode=DR)
                    else:
                        for i in range(2):
                            nc.tensor.matmul(
                                psum[:],
                                w1_sb[:, ds(((ht * IT2 + it2) * 2 + i) * P, P)],
                                enc_sb[:, ds(2 * it2 + i, 1), :],
                                start=(it2 == 0 and i == 0),
                                stop=(it2 == IT2 - 1 and i == 1))

            # ---- overflow streams: shared remainder columns of all 4
            # rows. Emitted mostly before the row streams so every row's
            # full score segment is ready before its softmax tail; the
            # last two groups interleave with row0 so the overflow tanh
            # backlog doesn't stall row0's PSUM recycling.
            ovf = []
            for k, rk in enumerate(rem):
                ovf.append({
                    "k": k, "rk": rk, "w": BL * rk,
                    "psr": pscp.tile([P, main], F32, tag="psc",
                                     name=f"psr{k}"),
                    "pend": [], "thP": None,
                })

            def ovf_group(st, ht):
                k, w = st["k"], st["w"]
                par = parp.tile([P, w], F32, tag="par")
                main_mms(par, encr_sb[k], ht)
                tp = thp.tile([P, w], F32, tag=f"tpre{k}")
                nc.vector.tensor_add(tp[:], par[:],
                                     biasb_sb[k][:, ds(ht * w, w)])
                if ht % 2 == 0:
                    st["thP"] = thp.tile([P, 2 * w], SDT, tag=f"thr{k}",
                                         name=f"thr{k}")
                nc.scalar.activation(st["thP"][:, ds((ht % 2) * w, w)], tp[:],
                                     AF.Tanh, scale=ACT_SCALE)
                if ht % 2 == 1:
                    st["pend"].append((st["thP"], ht // 2))
                    if len(st["pend"]) > 1:
                        scores_mm(st["psr"], w, *st["pend"].pop(0))

            def ovf_finish(st, off):
                for e in st["pend"]:
                    scores_mm(st["psr"], st["w"], *e)
                st["pend"] = []
                for r in range(BL):
                    pos = r * LT + off
                    nc.vector.tensor_add(scores_sb[0:1, ds(pos, st["rk"])],
                                         st["psr"][0:1, ds(r * st["rk"], st["rk"])],
                                         pneg_sb[0:1, ds(pos, st["rk"])])

            # ---- main streams: per batch row, with a single pending-
            # scores queue across rows so row-end drains interleave with
            # the next row's matmuls instead of bubbling the PE.
            def tail(r, psc):
                nc.vector.tensor_add(scores_sb[0:1, ds(r * LT, main)],
                                     psc[0:1, 0:main],
                                     pneg_sb[0:1, ds(r * LT, main)])
                # |scores| <= 2^12 * 16; exp(2^-12 s - 40) never overflows
                # and softmax is shift-invariant -- no max-reduce needed.
                nc.scalar.activation(exps[0:1, ds(r * LT, LT)],
                                     scores_sb[0:1, ds(r * LT, LT)],
                                     AF.Exp, bias=c40[0:1, 0:1],
                                     scale=1.0 / W2_SCALE,
                                     accum_out=ssum[0:1, ds(r, 1)])
                nc.sync.dma_start(out_ext[r, :], exps[0:1, ds(r * LT, LT)])
                if r == BL - 1:
                    nc.sync.dma_start(osum_ext[:], ssum[0:1, :])

            pend = []

            def pop_scores():
                thP, htp, ppsc, prow = pend.pop(0)
                scores_mm(ppsc, main, thP, htp)
                if htp == HTP - 1:
                    tail(prow, ppsc)

            def row_group(r, ht, psc, box):
                pa = pap.tile([P, main], F32, tag="pa")
                main_mms(pa, encm_sb[r], ht)
                if ht % 2 == 0:
                    box["thP"] = thp.tile([P, 2 * main], SDT, tag="th",
                                          name="thmain")
                nc.scalar.activation(box["thP"][:, ds((ht % 2) * main, main)],
                                     pa[:], AF.Tanh,
                                     bias=bias_sb[:, ds(ht * BL + r, 1)],
                                     scale=ACT_SCALE)
                if ht % 2 == 1:
                    pend.append((box["thP"], ht // 2, psc, r))
                    if len(pend) > DELAY:
                        pop_scores()

            # earlier rem blocks run fully before the rows
            offs = []
            off = main
            for st in ovf:
                offs.append(off)
                off += st["rk"]
            for st, o in zip(ovf[:-1], offs[:-1]):
                for ht in range(HT):
                    ovf_group(st, ht)
                ovf_finish(st, o)
            last = ovf[-1] if ovf else None
            if last is not None:
                for ht in range(HT - 2):
                    ovf_group(last, ht)

            for r in range(BL):
                psc = pscp.tile([P, main], F32, tag="psc")
                box = {"thP": None}
                for ht in range(HT):
                    row_group(r, ht, psc, box)
                    if r == 0 and last is not None and ht < 2:
                        ovf_group(last, HT - 2 + ht)
                        if ht == 1:
                            ovf_finish(last, offs[-1])
            while pend:
                pop_scores()

    nc.compile()
    _cached[key] = nc
    return nc


def _to_dev_dtype(a):
    if USE_FP8:
        return np.clip(a, -240.0, 240.0).astype(F8NP)
    return a.astype(BF)


def kernel(hidden, encoder_outputs, mask, W1, b1, W2, b2):
    global LAST_RESULT

    mask = np.asarray(mask, dtype=bool)
    idx = [np.nonzero(~mask[b])[0] for b in range(B)]
    cnt = np.array([len(i) for i in idx])
    main, rem = _geometry(int(cnt.max()))
    LT = main + sum(rem)
    nc = _build(main, rem)

    enc = np.asarray(encoder_outputs, dtype=np.float32)
    enc_t = np.transpose(enc, (1, 2, 0))            # [B, Hin, S]
    W1 = np.asarray(W1, dtype=np.float32)
    w1e = W1[:, :HIN].T                              # [Hin, H]
    w1h = W1[:, HIN:]                                # [H, H]
    hb = (np.asarray(hidden, np.float32) @ w1h.T
          + np.asarray(b1, np.float32).reshape(1, H))  # [B, H]
    w2 = np.asarray(W2, dtype=np.float32).reshape(H)

    # W1_enc.T packed for DoubleRow: [p, ht, it2, two, m]
    w1s = (w1e * W_SCALE) if USE_FP8 else w1e
    w1p = _to_dev_dtype(w1s).reshape(IT2, 2, P, HT, P)
    w1p = np.ascontiguousarray(np.transpose(w1p, (2, 3, 0, 1, 4))).reshape(P, -1)

    # W2 stationary: fp8 ht-pairs [p, htp, two, m] or bf16 [p, ht, m]
    if USE_FP8:
        w2pad = np.zeros((P, HTP, 2, P), dtype=F8NP)
        w2pad[:, :, :, 0] = np.transpose(
            (w2 * W2_SCALE).reshape(HTP, 2, P), (2, 0, 1)).astype(F8NP)
    else:
        w2pad = np.zeros((P, HT, P), dtype=BF)
        w2pad[:, :, 0] = w2.reshape(HT, P).T
    w2pad = w2pad.reshape(P, HT * P)

    # packed enc per row + padneg
    encs = enc_t * ENC_SCALE if USE_FP8 else enc_t
    edt = F8NP if USE_FP8 else BF
    encm = np.zeros((B, HIN, main), dtype=edt)
    encr = [np.zeros((N_CORES, HIN, BL * rk), dtype=edt) for rk in rem]
    pneg = np.full((B, LT), np.float32(-1e30), dtype=np.float32)
    for b in range(B):
        c, rloc = divmod(b, BL)
        cols = _to_dev_dtype(encs[b][:, idx[b]])
        n = cnt[b]
        nm = min(n, main)
        encm[b, :, :nm] = cols[:, :nm]
        pneg[b, :n] = 0.0
        off = main
        for k, rk in enumerate(rem):
            if n > off:
                w = min(n - off, rk)
                encr[k][c, :, rloc * rk:rloc * rk + w] = cols[:, off:off + w]
            off += rk

    # [c, p, ht*BL] per-(h,b) bias
    biasT = np.ascontiguousarray(
        np.transpose(hb.reshape(N_CORES, BL, HT, P), (0, 3, 2, 1))
    ).reshape(N_CORES, P, HT * BL).astype(np.float32)
    # overflow broadcast bias [c, p, ht*(BL*rk)], pre-scaled by 1/ACT_SCALE
    biasb = []
    for k, rk in enumerate(rem):
        bb = np.transpose(hb.reshape(N_CORES, BL, HT, P), (0, 3, 2, 1))
        bb = np.repeat(bb[:, :, :, :, None], rk, axis=4)   # [c,p,ht,BL,rk]
        biasb.append(np.ascontiguousarray(
            bb.reshape(N_CORES, P, HT * BL * rk) / ACT_SCALE
        ).astype(BF))

    in_maps = []
    for c in range(N_CORES):
        sl = slice(c * BL, (c + 1) * BL)
        m = {
            "encm": np.ascontiguousarray(encm[sl]),
            "w1p": w1p,
            "biasT": biasT[c],
            "w2pad": w2pad,
            "padneg": np.ascontiguousarray(pneg[sl].reshape(-1)),
        }
        for k in range(len(rem)):
            m[f"encr{k}"] = np.ascontiguousarray(encr[k][c])
            m[f"biasb{k}"] = biasb[k][c]
        in_maps.append(m)

    res = run_bass_kernel_spmd(nc, in_maps, core_ids=list(range(N_CORES)))
    LAST_RESULT = res

    out = np.zeros((B, S), dtype=np.float32)
    for b in range(B):
        c, rloc = divmod(b, BL)
        row = res.results[c]["out"][rloc]
        s0 = np.float32(res.results[c]["osum"][rloc])
        out[b, idx[b]] = row[:cnt[b]] / s0
    return np.ascontiguousarray(out[:, None, :])


# revision 30
# speedup vs baseline: 1.0636x; 1.0636x over previous
"""Trainium2 Bass kernel for the attention-scoring MLP (nn_Attn):

    enc = encoder_outputs.transpose(1,0,2)          # [B,S,Hin]
    a1  = tanh(enc @ W1_enc.T + hidden @ W1_hid.T + b1)
    s   = a1 @ W2[0] (+ b2 -- dropped: softmax shift-invariant)
    s   = where(mask, -inf, s)
    out = softmax(s, axis=-1)[:, None, :]           # [B,1,S]

Strategy (v6):
  * Data-parallel over batch B=32 across 8 NeuronCores (4 rows each),
    weights replicated, no collectives.
  * Mask packing: masked positions get attn == 0 exactly, so only the
    ~50% unmasked columns of enc are shipped/computed. Host packs each
    row's unmasked columns; device computes scores+softmax on the packed
    stream; host scatters back to [B,1,S] with zeros. Geometry: each row
    contributes a MAIN-wide stream (<=448) plus its overflow columns;
    the 4 rows' overflows are batched into shared overflow streams so
    every matmul keeps free-dim >= 256 (tiny-FD matmuls are LDWEIGHTS-
    bound and waste the PE).
  * Everything on the PE runs as fp8 (e4m3) DoubleRow matmuls -- two
    128-deep k-slices per instruction, 2x bf16 throughput, and no
    perf-mode switches (each switch costs a pipeline flush):
      - enc @ W1_enc.T: enc x32, W1 x2^13 host-side (clears fp8
        subnormals); the 2^-18 compensation rides the tanh scale port.
      - scores: tanh outputs are written as fp8; W2 is scaled x2^12 and
        contracted over ht-PAIRS (stationary [128,2,128], col 0 = w2);
        the 2^-12 compensation rides the exp scale port.
    The hidden @ W1_hid.T + b1 term (0.1% of FLOPs) is folded host-side
    into the per-(h,b) tanh bias: row streams use the activation's
    per-partition bias port; overflow streams (mixed rows per tile) get
    a host-precomputed broadcast bias added on the otherwise-idle DVE,
    pre-scaled by 2^18 so the tanh scale still matches.
  * Softmax per row on-device: exp(2^-12 s - 40) with accumulate,
    reciprocal, scale, DMA out the packed attn row.
"""

import numpy as np
import ml_dtypes

import concourse.bass as bass
import concourse.tile as tile
from concourse import bacc, mybir
from concourse.bass import ds
from concourse.bass_utils import run_bass_kernel_spmd

N_CORES = 8
B, S, HIN, H = 32, 1024, 1024, 1024
BL = B // N_CORES          # local batch rows per core
P = 128                    # partitions
IT = HIN // P              # 128-deep contraction tiles
IT2 = IT // 2              # DoubleRow pair tiles
HT = H // P                # output-feature tiles
HTP = HT // 2              # ht pairs for the DR scores contraction
F32 = mybir.dt.float32
BF16 = mybir.dt.bfloat16
F8 = mybir.dt.float8e4
AF = mybir.ActivationFunctionType
DR = mybir.MatmulPerfMode.DoubleRow
BF = ml_dtypes.bfloat16
F8NP = ml_dtypes.float8_e4m3

USE_FP8 = True
ENC_SCALE = 32.0
W_SCALE = float(2.0 ** 13)
ACT_SCALE = float(1.0 / (ENC_SCALE * W_SCALE)) if USE_FP8 else 1.0
W2_SCALE = float(2.0 ** 12) if USE_FP8 else 1.0
DELAY = 2                  # pending tanh->scores pipeline depth (ht pairs)

_cached = {}               # (main, rem) -> compiled Bacc
LAST_RESULT = None         # BassKernelResults of the most recent run


def _geometry(lmax: int):
    """MAIN width per row plus overflow-block widths (multiples of 16)."""
    rnd = lambda x: ((x + 15) // 16) * 16
    if lmax <= 448:
        return rnd(lmax), ()
    rem = []
    off = 448
    while off < lmax:
        rem.append(min(128, rnd(lmax - off)))
        off += rem[-1]
    return 448, tuple(rem)


def _build(main: int, rem: tuple):
    key = (main, rem)
    if key in _cached:
        return _cached[key]

    LT = main + sum(rem)
    EDT = F8 if USE_FP8 else BF16
    SDT = F8 if USE_FP8 else BF16     # scores-matmul operand dtype

    nc = bacc.Bacc("TRN2", target_bir_lowering=False, debug=False,
                   num_devices=N_CORES)

    encm_ext = nc.dram_tensor("encm", [BL, HIN, main], EDT, kind="ExternalInput").ap()
    encr_ext = [
        nc.dram_tensor(f"encr{k}", [HIN, BL * rk], EDT, kind="ExternalInput").ap()
        for k, rk in enumerate(rem)
    ]
    w1_ext = nc.dram_tensor("w1p", [P, HT * IT * P], EDT, kind="ExternalInput").ap()
    bias_ext = nc.dram_tensor("biasT", [P, HT * BL], F32, kind="ExternalInput").ap()
    # overflow broadcast bias, pre-scaled by 1/ACT_SCALE (bf16: half the
    # wire bytes; the DVE add upcasts)
    biasb_ext = [
        nc.dram_tensor(f"biasb{k}", [P, HT * BL * rk], BF16, kind="ExternalInput").ap()
        for k, rk in enumerate(rem)
    ]
    # W2 stationary: fp8 ht-pairs [p, htp, two, m] (col 0 = w2*2^12), or
    # bf16 [p, ht, m] for the fallback path
    w2_ext = nc.dram_tensor("w2pad", [P, HT * P], SDT, kind="ExternalInput").ap()
    pneg_ext = nc.dram_tensor("padneg", [BL * LT], F32, kind="ExternalInput").ap()
    # out rows are unnormalized exp(s - 40); osum has the row sums
    # (host divides during the scatter -- keeps recip+mul off the tail)
    out_ext = nc.dram_tensor("out", [BL, LT], F32, kind="ExternalOutput").ap()
    osum_ext = nc.dram_tensor("osum", [BL], F32, kind="ExternalOutput").ap()

    def pair2(ap2d):
        return ap2d.rearrange("p (two m) -> p two m", two=2)

    def wpair(w_sb, ht, it2):
        return pair2(w_sb[:, ds((ht * IT2 + it2) * 2 * P, 2 * P)])

    with tile.TileContext(nc) as tc:
        with (
            tc.tile_pool(name="consts", bufs=1) as consts,
            tc.tile_pool(name="thp", bufs=4) as thp,
            tc.tile_pool(name="pap", bufs=3, space="PSUM") as pap,
            tc.tile_pool(name="parp", bufs=3, space="PSUM") as parp,
            tc.tile_pool(name="pscp", bufs=2, space="PSUM") as pscp,
        ):
            # ---- PE warmup: junk matmuls with no DMA deps so the HAM
            # clock-gate ramps while the first DMAs land.
            warm_sb = consts.tile([P, 512], BF16)
            nc.gpsimd.memset(warm_sb[:], 0.0)
            for _ in range(6):
                warm_ps = pap.tile([P, main], F32, tag="pa")
                nc.tensor.matmul(warm_ps[:], warm_sb[:, 0:P],
                                 warm_sb[:, 0:main], start=True, stop=True)

            # ---- resident weights/constants.
            # Emission order = ring service order: first-needed first.
            # w1 split per ht so the first overflow matmul only waits for
            # one eighth of the 1MB weight load.
            w1_sb = consts.tile([P, HT * IT * P], EDT)
            nc.sync.dma_start(w1_sb[:, ds(0, IT * P)], w1_ext[:, ds(0, IT * P)])
            encr_sb, biasb_sb = [], []
            for k, rk in enumerate(rem):
                w = BL * rk
                e = consts.tile([P, IT, w], EDT, tag=f"encr{k}")
                h = IT // 2
                for half in range(2):
                    nc.scalar.dma_start(
                        e[:, ds(half * h, h), :],
                        encr_ext[k][ds(half * h * P, h * P), :].rearrange(
                            "(it p) n -> p it n", p=P))
                encr_sb.append(e)
                bb = consts.tile([P, HT * w], BF16, tag=f"biasb{k}")
                nc.sync.dma_start(bb[:], biasb_ext[k][:, :])
                biasb_sb.append(bb)
            for ht in range(1, HT):
                nc.sync.dma_start(w1_sb[:, ds(ht * IT * P, IT * P)],
                                  w1_ext[:, ds(ht * IT * P, IT * P)])
            w2_sb = consts.tile([P, HT * P], SDT)
            nc.sync.dma_start(w2_sb[:], w2_ext[:, :])
            pneg_sb = consts.tile([1, BL * LT], F32)
            nc.scalar.dma_start(pneg_sb[:], pneg_ext[:])
            bias_sb = consts.tile([P, HT * BL], F32)
            nc.scalar.dma_start(bias_sb[:], bias_ext[:, :])
            encm_sb = []
            for r in range(BL):
                e = consts.tile([P, IT, main], EDT, tag=f"encm{r}")
                eng = nc.scalar if r < 2 else nc.sync
                eng.dma_start(
                    e[:], encm_ext[r].rearrange("(it p) n -> p it n", p=P))
                encm_sb.append(e)

            scores_sb = consts.tile([1, BL * LT], F32)
            c40 = consts.tile([1, 1], F32)
            nc.gpsimd.memset(c40[:], -40.0)
            exps = consts.tile([1, BL * LT], F32)
            ssum = consts.tile([1, BL], F32)

            def scores_mm(psum, n, thP, htp):
                if USE_FP8:
                    nc.tensor.matmul(psum[:, 0:n],
                                     pair2(w2_sb[:, ds(htp * 2 * P, 2 * P)]),
                                     pair2(thP[:]), start=(htp == 0),
                                     stop=(htp == HTP - 1), perf_mode=DR)
                else:
                    for i in range(2):
                        ht = 2 * htp + i
                        nc.tensor.matmul(psum[:, 0:n],
                                         w2_sb[:, ds(ht * P, P)],
                                         thP[:, ds(i * n, n)],
                                         start=(ht == 0), stop=(ht == HT - 1))

            def main_mms(psum, enc_sb, ht):
                for it2 in range(IT2):
                    if USE_FP8:
                        nc.tensor.matmul(
                            psum[:], wpair(w1_sb, ht, it2),
                            enc_sb[:, ds(2 * it2, 2), :],
                            start=(it2 == 0), stop=(it2 == IT2 - 1),
                            perf_mode=DR)
                    else:
                        for i in range(2):
                            nc.tensor.matmul(
                                psum[:],
                                w1_sb[:, ds(((ht * IT2 + it2) * 2 + i) * P, P)],
                                enc_sb[:, ds(2 * it2 + i, 1), :],
                                start=(it2 == 0 and i == 0),
                                stop=(it2 == IT2 - 1 and i == 1))

            # ---- overflow streams: shared remainder columns of all 4
            # rows. Emitted mostly before the row streams so every row's
            # full score segment is ready before its softmax tail; the
            # last two groups interleave with row0 so the overflow tanh
            # backlog doesn't stall row0's PSUM recycling.
            ovf = []
            for k, rk in enumerate(rem):
                ovf.append({
                    "k": k, "rk": rk, "w": BL * rk,
                    "psr": pscp.tile([P, main], F32, tag="psc",
                                     name=f"psr{k}"),
                    "pend": [], "thP": None,
                })

            def ovf_group(st, ht):
                k, w = st["k"], st["w"]
                par = parp.tile([P, w], F32, tag="par")
                main_mms(par, encr_sb[k], ht)
                tp = thp.tile([P, w], F32, tag=f"tpre{k}")
                nc.vector.tensor_add(tp[:], par[:],
                                     biasb_sb[k][:, ds(ht * w, w)])
                if ht % 2 == 0:
                    st["thP"] = thp.tile([P, 2 * w], SDT, tag=f"thr{k}",
                                         name=f"thr{k}")
                nc.scalar.activation(st["thP"][:, ds((ht % 2) * w, w)], tp[:],
                                     AF.Tanh, scale=ACT_SCALE)
                if ht % 2 == 1:
                    st["pend"].append((st["thP"], ht // 2))
                    if len(st["pend"]) > 1:
                        scores_mm(st["psr"], w, *st["pend"].pop(0))

            def ovf_finish(st, off):
                for e in st["pend"]:
                    scores_mm(st["psr"], st["w"], *e)
                st["pend"] = []
                for r in range(BL):
                    pos = r * LT + off
                    nc.vector.tensor_add(scores_sb[0:1, ds(pos, st["rk"])],
                                         st["psr"][0:1, ds(r * st["rk"], st["rk"])],
                                         pneg_sb[0:1, ds(pos, st["rk"])])

            # ---- main streams: per batch row, with a single pending-
            # scores queue across rows so row-end drains interleave with
            # the next row's matmuls instead of bubbling the PE.
            def tail(r, psc):
                nc.vector.tensor_add(scores_sb[0:1, ds(r * LT, main)],
                                     psc[0:1, 0:main],
                                     pneg_sb[0:1, ds(r * LT, main)])
                # |scores| <= 2^12 * 16; exp(2^-12 s - 40) never overflows
                # and softmax is shift-invariant -- no max-reduce needed.
                nc.scalar.activation(exps[0:1, ds(r * LT, LT)],
                                     scores_sb[0:1, ds(r * LT, LT)],
                                     AF.Exp, bias=c40[0:1, 0:1],
                                     scale=1.0 / W2_SCALE,
                                     accum_out=ssum[0:1, ds(r, 1)])
                nc.sync.dma_start(out_ext[r, :], exps[0:1, ds(r * LT, LT)])
                if r == BL - 1:
                    nc.sync.dma_start(osum_ext[:], ssum[0:1, :])

            pend = []

            def pop_scores():
                thP, htp, ppsc, prow = pend.pop(0)
                scores_mm(ppsc, main, thP, htp)
                if htp == HTP - 1:
                    tail(prow, ppsc)

            def row_group(r, ht, psc, box):
                pa = pap.tile([P, main], F32, tag="pa")
                main_mms(pa, encm_sb[r], ht)
                if ht % 2 == 0:
                    box["thP"] = thp.tile([P, 2 * main], SDT, tag="th",
                                          name="thmain")
                nc.scalar.activation(box["thP"][:, ds((ht % 2) * main, main)],
                                     pa[:], AF.Tanh,
                                     bias=bias_sb[:, ds(ht * BL + r, 1)],
                                     scale=ACT_SCALE)
                if ht % 2 == 1:
                    pend.append((box["thP"], ht // 2, psc, r))
                    if len(pend) > DELAY:
                        pop_scores()

            # earlier rem blocks run fully before the rows
            offs = []
            off = main
            for st in ovf:
                offs.append(off)
                off += st["rk"]
            for st, o in zip(ovf[:-1], offs[:-1]):
                for ht in range(HT):
                    ovf_group(st, ht)
                ovf_finish(st, o)
            last = ovf[-1] if ovf else None
            if last is not None:
                for ht in range(HT - 2):
                    ovf_group(last, ht)

            for r in range(BL):
                psc = pscp.tile([P, main], F32, tag="psc")
                box = {"thP": None}
                for ht in range(HT):
                    row_group(r, ht, psc, box)
                    if r == 0 and last is not None and ht < 2:
                        ovf_group(last, HT - 2 + ht)
                        if ht == 1:
                            ovf_finish(last, offs[-1])
            while pend:
                pop_scores()

    nc.compile()
    _cached[key] = nc
    return nc


def _to_dev_dtype(a):
    if USE_FP8:
        return np.clip(a, -240.0, 240.0).astype(F8NP)
    return a.astype(BF)


def kernel(hidden, encoder_outputs, mask, W1, b1, W2, b2):
    global LAST_RESULT

    mask = np.asarray(mask, dtype=bool)
    idx = [np.nonzero(~mask[b])[0] for b in range(B)]
    cnt = np.array([len(i) for i in idx])
    main, rem = _geometry(int(cnt.max()))
    LT = main + sum(rem)
    nc = _build(main, rem)

    enc = np.asarray(encoder_outputs, dtype=np.float32)
    enc_t = np.transpose(enc, (1, 2, 0))            # [B, Hin, S]
    W1 = np.asarray(W1, dtype=np.float32)
    w1e = W1[:, :HIN].T                              # [Hin, H]
    w1h = W1[:, HIN:]                                # [H, H]
    hb = (np.asarray(hidden, np.float32) @ w1h.T
          + np.asarray(b1, np.float32).reshape(1, H))  # [B, H]
    w2 = np.asarray(W2, dtype=np.float32).reshape(H)

    # W1_enc.T packed for DoubleRow: [p, ht, it2, two, m]
    w1s = (w1e * W_SCALE) if USE_FP8 else w1e
    w1p = _to_dev_dtype(w1s).reshape(IT2, 2, P, HT, P)
    w1p = np.ascontiguousarray(np.transpose(w1p, (2, 3, 0, 1, 4))).reshape(P, -1)

    # W2 stationary: fp8 ht-pairs [p, htp, two, m] or bf16 [p, ht, m]
    if USE_FP8:
        w2pad = np.zeros((P, HTP, 2, P), dtype=F8NP)
        w2pad[:, :, :, 0] = np.transpose(
            (w2 * W2_SCALE).reshape(HTP, 2, P), (2, 0, 1)).astype(F8NP)
    else:
        w2pad = np.zeros((P, HT, P), dtype=BF)
        w2pad[:, :, 0] = w2.reshape(HT, P).T
    w2pad = w2pad.reshape(P, HT * P)

    # packed enc per row + padneg
    encs = enc_t * ENC_SCALE if USE_FP8 else enc_t
    edt = F8NP if USE_FP8 else BF
    encm = np.zeros((B, HIN, main), dtype=edt)
    encr = [np.zeros((N_CORES, HIN, BL * rk), dtype=edt) for rk in rem]
    pneg = np.full((B, LT), np.float32(-1e30), dtype=np.float32)
    for b in range(B):
        c, rloc = divmod(b, BL)
        cols = _to_dev_dtype(encs[b][:, idx[b]])
        n = cnt[b]
        nm = min(n, main)
        encm[b, :, :nm] = cols[:, :nm]
        pneg[b, :n] = 0.0
        off = main
        for k, rk in enumerate(rem):
            if n > off:
                w = min(n - off, rk)
                encr[k][c, :, rloc * rk:rloc * rk + w] = cols[:, off:off + w]
            off += rk

    # [c, p, ht*BL] per-(h,b) bias
    biasT = np.ascontiguousarray(
        np.transpose(hb.reshape(N_CORES, BL, HT, P), (0, 3, 2, 1))
    ).reshape(N_CORES, P, HT * BL).astype(np.float32)
    # overflow broadcast bias [c, p, ht*(BL*rk)], pre-scaled by 1/ACT_SCALE
    biasb = []
    for k, rk in enumerate(rem):
        bb = np.transpose(hb.reshape(N_CORES, BL, HT, P), (0, 3, 2, 1))
        bb = np.repeat(bb[:, :, :, :, None], rk, axis=4)   # [c,p,ht,BL,rk]
        biasb.append(np.ascontiguousarray(
            bb.reshape(N_CORES, P, HT * BL * rk) / ACT_SCALE
        ).astype(BF))

    in_maps = []
    for c in range(N_CORES):
        sl = slice(c * BL, (c + 1) * BL)
        m = {
            "encm": np.ascontiguousarray(encm[sl]),
            "w1p": w1p,
            "biasT": biasT[c],
            "w2pad": w2pad,
            "padneg": np.ascontiguousarray(pneg[sl].reshape(-1)),
        }
        for k in range(len(rem)):
            m[f"encr{k}"] = np.ascontiguousarray(encr[k][c])
            m[f"biasb{k}"] = biasb[k][c]
        in_maps.append(m)

    res = run_bass_kernel_spmd(nc, in_maps, core_ids=list(range(N_CORES)))
    LAST_RESULT = res

    out = np.zeros((B, S), dtype=np.float32)
    for b in range(B):
        c, rloc = divmod(b, BL)
        row = res.results[c]["out"][rloc]
        s0 = np.float32(res.results[c]["osum"][rloc])
        out[b, idx[b]] = row[:cnt[b]] / s0
    return np.ascontiguousarray(out[:, None, :])


# revision 31
# speedup vs baseline: 1.0667x; 1.0029x over previous
"""Trainium2 Bass kernel for the attention-scoring MLP (nn_Attn):

    enc = encoder_outputs.transpose(1,0,2)          # [B,S,Hin]
    a1  = tanh(enc @ W1_enc.T + hidden @ W1_hid.T + b1)
    s   = a1 @ W2[0] (+ b2 -- dropped: softmax shift-invariant)
    s   = where(mask, -inf, s)
    out = softmax(s, axis=-1)[:, None, :]           # [B,1,S]

Strategy (v6):
  * Data-parallel over batch B=32 across 8 NeuronCores (4 rows each),
    weights replicated, no collectives.
  * Mask packing: masked positions get attn == 0 exactly, so only the
    ~50% unmasked columns of enc are shipped/computed. Host packs each
    row's unmasked columns; device computes scores+softmax on the packed
    stream; host scatters back to [B,1,S] with zeros. Geometry: each row
    contributes a MAIN-wide stream (<=448) plus its overflow columns;
    the 4 rows' overflows are batched into shared overflow streams so
    every matmul keeps free-dim >= 256 (tiny-FD matmuls are LDWEIGHTS-
    bound and waste the PE).
  * Everything on the PE runs as fp8 (e4m3) DoubleRow matmuls -- two
    128-deep k-slices per instruction, 2x bf16 throughput, and no
    perf-mode switches (each switch costs a pipeline flush):
      - enc @ W1_enc.T: enc x32, W1 x2^13 host-side (clears fp8
        subnormals); the 2^-18 compensation rides the tanh scale port.
      - scores: tanh outputs are written as fp8; W2 is scaled x2^12 and
        contracted over ht-PAIRS (stationary [128,2,128], col 0 = w2);
        the 2^-12 compensation rides the exp scale port.
    The hidden @ W1_hid.T + b1 term (0.1% of FLOPs) is folded host-side
    into the per-(h,b) tanh bias: row streams use the activation's
    per-partition bias port; overflow streams (mixed rows per tile) get
    a host-precomputed broadcast bias added on the otherwise-idle DVE,
    pre-scaled by 2^18 so the tanh scale still matches.
  * Softmax per row on-device: exp(2^-12 s - 40) with accumulate,
    reciprocal, scale, DMA out the packed attn row.
"""

import numpy as np
import ml_dtypes

import concourse.bass as bass
import concourse.tile as tile
from concourse import bacc, mybir
from concourse.bass import ds
from concourse.bass_utils import run_bass_kernel_spmd

N_CORES = 8
B, S, HIN, H = 32, 1024, 1024, 1024
BL = B // N_CORES          # local batch rows per core
P = 128                    # partitions
IT = HIN // P              # 128-deep contraction tiles
IT2 = IT // 2              # DoubleRow pair tiles
HT = H // P                # output-feature tiles
HTP = HT // 2              # ht pairs for the DR scores contraction
F32 = mybir.dt.float32
BF16 = mybir.dt.bfloat16
F8 = mybir.dt.float8e4
AF = mybir.ActivationFunctionType
DR = mybir.MatmulPerfMode.DoubleRow
BF = ml_dtypes.bfloat16
F8NP = ml_dtypes.float8_e4m3

USE_FP8 = True
ENC_SCALE = 32.0
W_SCALE = float(2.0 ** 13)
ACT_SCALE = float(1.0 / (ENC_SCALE * W_SCALE)) if USE_FP8 else 1.0
W2_SCALE = float(2.0 ** 12) if USE_FP8 else 1.0
DELAY = 2                  # pending tanh->scores pipeline depth (ht pairs)

_cached = {}               # (main, rem) -> compiled Bacc
LAST_RESULT = None         # BassKernelResults of the most recent run


def _geometry(lmax: int):
    """MAIN width per row plus overflow-block widths (multiples of 16)."""
    rnd = lambda x: ((x + 15) // 16) * 16
    if lmax <= 448:
        return rnd(lmax), ()
    rem = []
    off = 448
    while off < lmax:
        rem.append(min(128, rnd(lmax - off)))
        off += rem[-1]
    return 448, tuple(rem)


def _build(main: int, rem: tuple):
    key = (main, rem)
    if key in _cached:
        return _cached[key]

    LT = main + sum(rem)
    EDT = F8 if USE_FP8 else BF16
    SDT = F8 if USE_FP8 else BF16     # scores-matmul operand dtype

    nc = bacc.Bacc("TRN2", target_bir_lowering=False, debug=False,
                   num_devices=N_CORES)

    encm_ext = nc.dram_tensor("encm", [BL, HIN, main], EDT, kind="ExternalInput").ap()
    encr_ext = [
        nc.dram_tensor(f"encr{k}", [HIN, BL * rk], EDT, kind="ExternalInput").ap()
        for k, rk in enumerate(rem)
    ]
    w1_ext = nc.dram_tensor("w1p", [P, HT * IT * P], EDT, kind="ExternalInput").ap()
    bias_ext = nc.dram_tensor("biasT", [P, HT * BL], F32, kind="ExternalInput").ap()
    # overflow broadcast bias, pre-scaled by 1/ACT_SCALE (bf16: half the
    # wire bytes; the DVE add upcasts)
    biasb_ext = [
        nc.dram_tensor(f"biasb{k}", [P, HT * BL * rk], BF16, kind="ExternalInput").ap()
        for k, rk in enumerate(rem)
    ]
    # W2 stationary: fp8 ht-pairs [p, htp, two, m] (col 0 = w2*2^12), or
    # bf16 [p, ht, m] for the fallback path
    w2_ext = nc.dram_tensor("w2pad", [P, HT * P], SDT, kind="ExternalInput").ap()
    pneg_ext = nc.dram_tensor("padneg", [BL * LT], F32, kind="ExternalInput").ap()
    # out rows are unnormalized exp(s - 40); osum has the row sums
    # (host divides during the scatter -- keeps recip+mul off the tail)
    out_ext = nc.dram_tensor("out", [BL, LT], F32, kind="ExternalOutput").ap()
    osum_ext = nc.dram_tensor("osum", [BL], F32, kind="ExternalOutput").ap()

    def pair2(ap2d):
        return ap2d.rearrange("p (two m) -> p two m", two=2)

    def wpair(w_sb, ht, it2):
        return pair2(w_sb[:, ds((ht * IT2 + it2) * 2 * P, 2 * P)])

    with tile.TileContext(nc) as tc:
        with (
            tc.tile_pool(name="consts", bufs=1) as consts,
            tc.tile_pool(name="thp", bufs=4) as thp,
            tc.tile_pool(name="pap", bufs=3, space="PSUM") as pap,
            tc.tile_pool(name="parp", bufs=3, space="PSUM") as parp,
            tc.tile_pool(name="pscp", bufs=2, space="PSUM") as pscp,
        ):
            # ---- PE warmup: junk matmuls with no DMA deps so the HAM
            # clock-gate ramps while the first DMAs land.
            warm_sb = consts.tile([P, 512], BF16)
            nc.gpsimd.memset(warm_sb[:], 0.0)
            for _ in range(6):
                warm_ps = pap.tile([P, main], F32, tag="pa")
                nc.tensor.matmul(warm_ps[:], warm_sb[:, 0:P],
                                 warm_sb[:, 0:main], start=True, stop=True)

            # ---- resident weights/constants.
            # Emission order = ring service order: first-needed first.
            # w1 split per ht so the first overflow matmul only waits for
            # one eighth of the 1MB weight load.
            w1_sb = consts.tile([P, HT * IT * P], EDT)
            nc.sync.dma_start(w1_sb[:, ds(0, IT * P)], w1_ext[:, ds(0, IT * P)])
            encr_sb, biasb_sb = [], []
            for k, rk in enumerate(rem):
                w = BL * rk
                e = consts.tile([P, IT, w], EDT, tag=f"encr{k}")
                h = IT // 2
                # halves on parallel rings (descriptor-rate limited, so one
                # queue serializes them); the Pool ring is otherwise idle
                nc.scalar.dma_start(
                    e[:, ds(0, h), :],
                    encr_ext[k][ds(0, h * P), :].rearrange(
                        "(it p) n -> p it n", p=P))
                nc.gpsimd.dma_start(
                    e[:, ds(h, h), :],
                    encr_ext[k][ds(h * P, h * P), :].rearrange(
                        "(it p) n -> p it n", p=P))
                encr_sb.append(e)
                bb = consts.tile([P, HT * w], BF16, tag=f"biasb{k}")
                nc.sync.dma_start(bb[:], biasb_ext[k][:, :])
                biasb_sb.append(bb)
            for ht in range(1, HT):
                nc.sync.dma_start(w1_sb[:, ds(ht * IT * P, IT * P)],
                                  w1_ext[:, ds(ht * IT * P, IT * P)])
            w2_sb = consts.tile([P, HT * P], SDT)
            nc.sync.dma_start(w2_sb[:], w2_ext[:, :])
            pneg_sb = consts.tile([1, BL * LT], F32)
            nc.scalar.dma_start(pneg_sb[:], pneg_ext[:])
            bias_sb = consts.tile([P, HT * BL], F32)
            nc.scalar.dma_start(bias_sb[:], bias_ext[:, :])
            encm_sb = []
            for r in range(BL):
                e = consts.tile([P, IT, main], EDT, tag=f"encm{r}")
                eng = nc.scalar if r < 2 else nc.sync
                eng.dma_start(
                    e[:], encm_ext[r].rearrange("(it p) n -> p it n", p=P))
                encm_sb.append(e)

            scores_sb = consts.tile([1, BL * LT], F32)
            c40 = consts.tile([1, 1], F32)
            nc.gpsimd.memset(c40[:], -40.0)
            exps = consts.tile([1, BL * LT], F32)
            ssum = consts.tile([1, BL], F32)

            def scores_mm(psum, n, thP, htp):
                if USE_FP8:
                    nc.tensor.matmul(psum[:, 0:n],
                                     pair2(w2_sb[:, ds(htp * 2 * P, 2 * P)]),
                                     pair2(thP[:]), start=(htp == 0),
                                     stop=(htp == HTP - 1), perf_mode=DR)
                else:
                    for i in range(2):
                        ht = 2 * htp + i
                        nc.tensor.matmul(psum[:, 0:n],
                                         w2_sb[:, ds(ht * P, P)],
                                         thP[:, ds(i * n, n)],
                                         start=(ht == 0), stop=(ht == HT - 1))

            def main_mms(psum, enc_sb, ht):
                for it2 in range(IT2):
                    if USE_FP8:
                        nc.tensor.matmul(
                            psum[:], wpair(w1_sb, ht, it2),
                            enc_sb[:, ds(2 * it2, 2), :],
                            start=(it2 == 0), stop=(it2 == IT2 - 1),
                            perf_mode=DR)
                    else:
                        for i in range(2):
                            nc.tensor.matmul(
                                psum[:],
                                w1_sb[:, ds(((ht * IT2 + it2) * 2 + i) * P, P)],
                                enc_sb[:, ds(2 * it2 + i, 1), :],
                                start=(it2 == 0 and i == 0),
                                stop=(it2 == IT2 - 1 and i == 1))

            # ---- overflow streams: shared remainder columns of all 4
            # rows. Emitted mostly before the row streams so every row's
            # full score segment is ready before its softmax tail; the
            # last two groups interleave with row0 so the overflow tanh
            # backlog doesn't stall row0's PSUM recycling.
            ovf = []
            for k, rk in enumerate(rem):
                ovf.append({
                    "k": k, "rk": rk, "w": BL * rk,
                    "psr": pscp.tile([P, main], F32, tag="psc",
                                     name=f"psr{k}"),
                    "pend": [], "thP": None,
                })

            def ovf_group(st, ht):
                k, w = st["k"], st["w"]
                par = parp.tile([P, w], F32, tag="par")
                main_mms(par, encr_sb[k], ht)
                tp = thp.tile([P, w], F32, tag=f"tpre{k}")
                nc.vector.tensor_add(tp[:], par[:],
                                     biasb_sb[k][:, ds(ht * w, w)])
                if ht % 2 == 0:
                    st["thP"] = thp.tile([P, 2 * w], SDT, tag=f"thr{k}",
                                         name=f"thr{k}")
                nc.scalar.activation(st["thP"][:, ds((ht % 2) * w, w)], tp[:],
                                     AF.Tanh, scale=ACT_SCALE)
                if ht % 2 == 1:
                    st["pend"].append((st["thP"], ht // 2))
                    if len(st["pend"]) > 1:
                        scores_mm(st["psr"], w, *st["pend"].pop(0))

            def ovf_finish(st, off):
                for e in st["pend"]:
                    scores_mm(st["psr"], st["w"], *e)
                st["pend"] = []
                for r in range(BL):
                    pos = r * LT + off
                    nc.vector.tensor_add(scores_sb[0:1, ds(pos, st["rk"])],
                                         st["psr"][0:1, ds(r * st["rk"], st["rk"])],
                                         pneg_sb[0:1, ds(pos, st["rk"])])

            # ---- main streams: per batch row, with a single pending-
            # scores queue across rows so row-end drains interleave with
            # the next row's matmuls instead of bubbling the PE.
            def tail(r, psc):
                nc.vector.tensor_add(scores_sb[0:1, ds(r * LT, main)],
                                     psc[0:1, 0:main],
                                     pneg_sb[0:1, ds(r * LT, main)])
                # |scores| <= 2^12 * 16; exp(2^-12 s - 40) never overflows
                # and softmax is shift-invariant -- no max-reduce needed.
                nc.scalar.activation(exps[0:1, ds(r * LT, LT)],
                                     scores_sb[0:1, ds(r * LT, LT)],
                                     AF.Exp, bias=c40[0:1, 0:1],
                                     scale=1.0 / W2_SCALE,
                                     accum_out=ssum[0:1, ds(r, 1)])
                nc.sync.dma_start(out_ext[r, :], exps[0:1, ds(r * LT, LT)])
                if r == BL - 1:
                    nc.sync.dma_start(osum_ext[:], ssum[0:1, :])

            pend = []

            def pop_scores():
                thP, htp, ppsc, prow = pend.pop(0)
                scores_mm(ppsc, main, thP, htp)
                if htp == HTP - 1:
                    tail(prow, ppsc)

            def row_group(r, ht, psc, box):
                pa = pap.tile([P, main], F32, tag="pa")
                main_mms(pa, encm_sb[r], ht)
                if ht % 2 == 0:
                    box["thP"] = thp.tile([P, 2 * main], SDT, tag="th",
                                          name="thmain")
                nc.scalar.activation(box["thP"][:, ds((ht % 2) * main, main)],
                                     pa[:], AF.Tanh,
                                     bias=bias_sb[:, ds(ht * BL + r, 1)],
                                     scale=ACT_SCALE)
                if ht % 2 == 1:
                    pend.append((box["thP"], ht // 2, psc, r))
                    if len(pend) > DELAY:
                        pop_scores()

            # earlier rem blocks run fully before the rows
            offs = []
            off = main
            for st in ovf:
                offs.append(off)
                off += st["rk"]
            for st, o in zip(ovf[:-1], offs[:-1]):
                for ht in range(HT):
                    ovf_group(st, ht)
                ovf_finish(st, o)
            last = ovf[-1] if ovf else None
            if last is not None:
                for ht in range(HT - 2):
                    ovf_group(last, ht)

            for r in range(BL):
                psc = pscp.tile([P, main], F32, tag="psc")
                box = {"thP": None}
                for ht in range(HT):
                    row_group(r, ht, psc, box)
                    if r == 0 and last is not None and ht < 2:
                        ovf_group(last, HT - 2 + ht)
                        if ht == 1:
                            ovf_finish(last, offs[-1])
            while pend:
                pop_scores()

    nc.compile()
    _cached[key] = nc
    return nc


def _to_dev_dtype(a):
    if USE_FP8:
        return np.clip(a, -240.0, 240.0).astype(F8NP)
    return a.astype(BF)


def kernel(hidden, encoder_outputs, mask, W1, b1, W2, b2):
    global LAST_RESULT

    mask = np.asarray(mask, dtype=bool)
    idx = [np.nonzero(~mask[b])[0] for b in range(B)]
    cnt = np.array([len(i) for i in idx])
    main, rem = _geometry(int(cnt.max()))
    LT = main + sum(rem)
    nc = _build(main, rem)

    enc = np.asarray(encoder_outputs, dtype=np.float32)
    enc_t = np.transpose(enc, (1, 2, 0))            # [B, Hin, S]
    W1 = np.asarray(W1, dtype=np.float32)
    w1e = W1[:, :HIN].T                              # [Hin, H]
    w1h = W1[:, HIN:]                                # [H, H]
    hb = (np.asarray(hidden, np.float32) @ w1h.T
          + np.asarray(b1, np.float32).reshape(1, H))  # [B, H]
    w2 = np.asarray(W2, dtype=np.float32).reshape(H)

    # W1_enc.T packed for DoubleRow: [p, ht, it2, two, m]
    w1s = (w1e * W_SCALE) if USE_FP8 else w1e
    w1p = _to_dev_dtype(w1s).reshape(IT2, 2, P, HT, P)
    w1p = np.ascontiguousarray(np.transpose(w1p, (2, 3, 0, 1, 4))).reshape(P, -1)

    # W2 stationary: fp8 ht-pairs [p, htp, two, m] or bf16 [p, ht, m]
    if USE_FP8:
        w2pad = np.zeros((P, HTP, 2, P), dtype=F8NP)
        w2pad[:, :, :, 0] = np.transpose(
            (w2 * W2_SCALE).reshape(HTP, 2, P), (2, 0, 1)).astype(F8NP)
    else:
        w2pad = np.zeros((P, HT, P), dtype=BF)
        w2pad[:, :, 0] = w2.reshape(HT, P).T
    w2pad = w2pad.reshape(P, HT * P)

    # packed enc per row + padneg
    encs = enc_t * ENC_SCALE if USE_FP8 else enc_t
    edt = F8NP if USE_FP8 else BF
    encm = np.zeros((B, HIN, main), dtype=edt)
    encr = [np.zeros((N_CORES, HIN, BL * rk), dtype=edt) for rk in rem]
    pneg = np.full((B, LT), np.float32(-1e30), dtype=np.float32)
    for b in range(B):
        c, rloc = divmod(b, BL)
        cols = _to_dev_dtype(encs[b][:, idx[b]])
        n = cnt[b]
        nm = min(n, main)
        encm[b, :, :nm] = cols[:, :nm]
        pneg[b, :n] = 0.0
        off = main
        for k, rk in enumerate(rem):
            if n > off:
                w = min(n - off, rk)
                encr[k][c, :, rloc * rk:rloc * rk + w] = cols[:, off:off + w]
            off += rk

    # [c, p, ht*BL] per-(h,b) bias
    biasT = np.ascontiguousarray(
        np.transpose(hb.reshape(N_CORES, BL, HT, P), (0, 3, 2, 1))
    ).reshape(N_CORES, P, HT * BL).astype(np.float32)
    # overflow broadcast bias [c, p, ht*(BL*rk)], pre-scaled by 1/ACT_SCALE
    biasb = []
    for k, rk in enumerate(rem):
        bb = np.transpose(hb.reshape(N_CORES, BL, HT, P), (0, 3, 2, 1))
        bb = np.repeat(bb[:, :, :, :, None], rk, axis=4)   # [c,p,ht,BL,rk]
        biasb.append(np.ascontiguousarray(
            bb.reshape(N_CORES, P, HT * BL * rk) / ACT_SCALE
        ).astype(BF))

    in_maps = []
    for c in range(N_CORES):
        sl = slice(c * BL, (c + 1) * BL)
        m = {
            "encm": np.ascontiguousarray(encm[sl]),
            "w1p": w1p,
            "biasT": biasT[c],
            "w2pad": w2pad,
            "padneg": np.ascontiguousarray(pneg[sl].reshape(-1)),
        }
        for k in range(len(rem)):
            m[f"encr{k}"] = np.ascontiguousarray(encr[k][c])
            m[f"biasb{k}"] = biasb[k][c]
        in_maps.append(m)

    res = run_bass_kernel_spmd(nc, in_maps, core_ids=list(range(N_CORES)))
    LAST_RESULT = res

    out = np.zeros((B, S), dtype=np.float32)
    for b in range(B):
        c, rloc = divmod(b, BL)
        row = res.results[c]["out"][rloc]
        s0 = np.float32(res.results[c]["osum"][rloc])
        out[b, idx[b]] = row[:cnt[b]] / s0
    return np.ascontiguousarray(out[:, None, :])
